# revision 36
# baseline (speedup 1.0000x reference)
"""Causal self-attention (B=4, T=2048, C=1024, H=16, Dh=64) on 8 trn2 NeuronCores.

Sharding: core i <-> (batch b = i//2, head-group g = i%2). Each core computes
8 heads of one batch end-to-end (qkv slice, causal attention, output
projection over its 512 features); the host sums the two head-group partials
per batch and adds bproj. No device collectives.

v2 layout notes:
- x arrives pre-transposed from host (xT [C, T] fp16) -- no PE transposes.
- Scores for the two heads of a pair are issued back-to-back with disjoint
  row-groups (lhsT base partitions 0 / 64), so the PE runs them concurrently
  in separate 64-row strips. Both land in one [128, 1024] PSUM tile (h0|h1),
  which gets a single fused Exp when the diagonal offset is 0.
- Causal mask applied as a post-exp fp16 multiply (cheaper than fp32 add).
- Softmax denominators come from an extra ones column interleaved into Wv
  (pv row 64); reciprocal is taken on [1,512] then broadcast to 64
  partitions via a DRAM bounce in fp16.
- pv PSUM tiles are released immediately via f16 copies so the next q-block's
  PV accumulation doesn't wait on the denominator DMA roundtrip.
- One fused projection over all 4 pairs, fp16 partial output, summed on host.
"""

import numpy as np

import concourse.bass as bass
import concourse.tile as tile
from concourse import bacc, mybir
from concourse.bass_utils import run_bass_kernel_spmd

F32 = mybir.dt.float32
F16 = mybir.dt.float16

N_CORES = 8
B, T, C = 4, 2048, 1024
NH_TOT, D = 16, 64
F = 512            # features per core (8 heads)
NH = 8             # local heads
NPAIR = 4          # head pairs (128 feats each)
CCH = C // 128     # 8 contraction chunks
NTT = T // 128     # 16 t tiles
NTB = T // 512     # 4 t blocks (qkv production)
NQB = T // 512     # 4 q blocks (attention)
VW = NH * (D + 1)  # 520: augmented v width
ADD = mybir.AluOpType.add
MULT = mybir.AluOpType.mult
EXP = mybir.ActivationFunctionType.Exp


def _emit(tc, aps):
    from contextlib import ExitStack
    nc = tc.nc
    xt, wq, wk, wva, bq, bk, wp = (
        aps["xt"], aps["wq"], aps["wk"], aps["wva"], aps["bq"], aps["bk"],
        aps["wp"])
    mask01 = aps["mask01"]
    out_d = aps["out"]

    ctx = ExitStack()
    pp_st = ctx.enter_context(tc.tile_pool(name="ps_st", bufs=2, space="PSUM"))
    pp_pv = ctx.enter_context(tc.tile_pool(name="ps_pv", bufs=1, space="PSUM"))
    pp_qk = ctx.enter_context(tc.tile_pool(name="ps_qk", bufs=2, space="PSUM"))
    po_xt = ctx.enter_context(tc.tile_pool(name="xT", bufs=1))
    po_v = ctx.enter_context(tc.tile_pool(name="v_all", bufs=1))
    po_mask = ctx.enter_context(tc.tile_pool(name="mask", bufs=1))
    po_wv = ctx.enter_context(tc.tile_pool(name="wv", bufs=1))
    po_wqk = ctx.enter_context(tc.tile_pool(name="wqk", bufs=2))
    po_qkt = ctx.enter_context(tc.tile_pool(name="qkT", bufs=2))
    po_bias = ctx.enter_context(tc.tile_pool(name="bias", bufs=1))
    po_exp = ctx.enter_context(tc.tile_pool(name="expT", bufs=3))
    po_yt = ctx.enter_context(tc.tile_pool(name="yT", bufs=4))
    po_ytu = ctx.enter_context(tc.tile_pool(name="ytu", bufs=3))
    po_den = ctx.enter_context(tc.tile_pool(name="den", bufs=4))
    po_rec = ctx.enter_context(tc.tile_pool(name="recip", bufs=4))
    po_wp = ctx.enter_context(tc.tile_pool(name="wp", bufs=1))
    po_ot = ctx.enter_context(tc.tile_pool(name="ot", bufs=3))
    po_dram = ctx.enter_context(tc.tile_pool(name="dram_scr", bufs=4,
                                             space="DRAM"))

    mask_sb = po_mask.tile([128, 512], F16, tag="mask")
    nc.sync.dma_start(out=mask_sb[:], in_=mask01[:])
    # bva broadcast to all 128 partitions straight from DRAM
    bva_bc = po_bias.tile([128, VW], F32, tag="bva_bc")
    bva2 = aps["bva2"]
    nc.sync.dma_start(out=bva_bc[:], in_=bass.AP(
        tensor=bva2.tensor, offset=bva2.offset,
        ap=[[0, 128]] + [list(a) for a in bva2.ap[1:]]))

    # xT: one [128, 8*T] tile (chunk-major), so a whole 2MB half loads as
    # a single fat DMA descriptor.
    xTt = po_xt.tile([128, CCH * T], F16, tag="xT", name="xTt")

    def xT(c):
        return xTt[:, c * T:(c + 1) * T]

    def _gather_cols(dst_tile, src, n_ch, blk_w, dst_stride, col0, eng):
        """One DMA: dst[:, c*dst_stride+j] = src[c*128+p, col0+j] over
        c in [0,n_ch), j in [0,blk_w)."""
        eng.dma_start(
            out=bass.AP(tensor=dst_tile.tensor, offset=dst_tile[:].offset,
                        ap=[list(dst_tile[:].ap[0]), [dst_stride, n_ch],
                            [1, blk_w]]),
            in_=bass.AP(tensor=src.tensor, offset=src.offset + col0,
                        ap=[[src.ap[-2][0], 128],
                            [src.ap[-2][0] * 128, n_ch], [1, blk_w]]))

    def load_xt_quarter(q, eng):
        # one descriptor per 512-col quarter (all 8 chunks) -- aligns with
        # exactly what qkv tb_q / v tt(4q..) / attention qb_q consume
        eng.dma_start(
            out=bass.AP(tensor=xTt.tensor, offset=xTt[:].offset + q * 512,
                        ap=[list(xTt[:].ap[0]), [T, CCH], [1, 512]]),
            in_=bass.AP(tensor=xt.tensor, offset=xt.offset + q * 512,
                        ap=[[T, 128], [T * 128, CCH], [1, 512]]))

    # ---- v production (augmented with ones columns, all 8 heads) ----
    v_all = [po_v.tile([128, VW], F16, tag=f"v{tt}", name=f"v{tt}")
             for tt in range(NTT)]
    wv_sb = po_wv.tile([128, CCH * VW], F16, tag="wv", name="wv_sb")

    def load_wv(eng):
        # single descriptor: wv_sb[:, c*520+j] = wva[c*128+p, j]
        _gather_cols(wv_sb, wva, CCH, VW, VW, 0, eng)

    def v_unit(tt):
        for half in range(2):
            cs = slice(half * 260, half * 260 + 260)
            ps = pp_qk.tile([128, 260], F32, tag="qk")
            for c in range(CCH):
                nc.tensor.matmul(
                    ps[:], xT(c)[:, tt * 128:(tt + 1) * 128],
                    wv_sb[:, c * VW + half * 260:c * VW + half * 260 + 260],
                    start=(c == 0), stop=(c == CCH - 1))
            nc.vector.tensor_add(v_all[tt][:, cs], ps[:], bva_bc[:, cs])

    # ---- qkv per pair ----
    def prep_qkv(pair, eng=None):
        eng = eng or nc.sync
        psl = pair * 128
        wqk = po_wqk.tile([128, CCH * 256], F16, tag="wqk", name="wqk")
        # two descriptors: wq chunks -> cols c*256+[0,128), wk -> +[128,256)
        _gather_cols(wqk, wq, CCH, 128, 256, psl, eng)
        eng.dma_start(
            out=bass.AP(tensor=wqk.tensor, offset=wqk[:].offset + 128,
                        ap=[list(wqk[:].ap[0]), [256, CCH], [1, 128]]),
            in_=bass.AP(tensor=wk.tensor, offset=wk.offset + psl,
                        ap=[[wk.ap[-2][0], 128],
                            [wk.ap[-2][0] * 128, CCH], [1, 128]]))
        bq_sb = po_bias.tile([128, 1], F32, tag=f"bq{pair}", name=f"bq{pair}")
        eng.dma_start(out=bq_sb[:], in_=bq[psl:psl + 128, :])
        bk_sb = po_bias.tile([128, 1], F32, tag=f"bk{pair}", name=f"bk{pair}")
        eng.dma_start(out=bk_sb[:], in_=bk[psl:psl + 128, :])
        qT = po_qkt.tile([128, T], F16, tag="qT", name="qT")
        kT = po_qkt.tile([128, T], F16, tag="kT", name="kT")
        return dict(wqk=wqk, bq=bq_sb, bk=bk_sb, qT=qT, kT=kT, pair=pair)

    def qkv_unit(st8, tb):
        tsl = slice(tb * 512, (tb + 1) * 512)
        wqk = st8["wqk"]
        psq = pp_qk.tile([128, 512], F32, tag="qk", name="psq")
        for c in range(CCH):
            nc.tensor.matmul(psq[:], wqk[:, c * 256:c * 256 + 128],
                             xT(c)[:, tsl],
                             start=(c == 0), stop=(c == CCH - 1))
        # psum*1/sqrt(D) + bq/sqrt(D)   (bq pre-scaled on host)
        nc.vector.tensor_scalar(
            out=st8["qT"][:, tsl], in0=psq[:], scalar1=0.125,
            scalar2=st8["bq"][:], op0=MULT, op1=ADD)
        psk = pp_qk.tile([128, 512], F32, tag="qk", name="psk")
        for c in range(CCH):
            nc.tensor.matmul(psk[:], wqk[:, c * 256 + 128:c * 256 + 256],
                             xT(c)[:, tsl],
                             start=(c == 0), stop=(c == CCH - 1))
        nc.vector.tensor_scalar(
            out=st8["kT"][:, tsl], in0=psk[:], scalar1=st8["bk"][:],
            scalar2=None, op0=ADD)

    # ---- attention: unit = one ktile-event of one q-block ----
    # Scores for both heads go into one [128, 1024] PSUM tile (h0 | h1),
    # issued adjacently with row-disjoint lhsT slices so the PE overlaps
    # them. exp(kt-1) is emitted between scores(kt) and pv(kt-1) so the
    # scalar engine runs one ktile ahead of the PV consumer.
    def attn_units(st8, yt):
        qT, kT = st8["qT"], st8["kT"]
        pair = st8["pair"]
        vsl0 = slice((pair * 2) * 65, (pair * 2) * 65 + 65)
        vsl1 = slice((pair * 2 + 1) * 65, (pair * 2 + 1) * 65 + 65)
        state = {"pend": None, "pv": None}

        def emit_exp_pv(st, kt, off, qb, nkt, pv):
            et = po_exp.tile([128, 1024], F16, tag="expT", name="et")
            if off == 0:
                nc.scalar.activation(et[:, 0:1024], st[:, 0:1024], EXP)
            else:
                # one activation over both heads' valid slices via a 3D AP
                stv = bass.AP(
                    tensor=st.tensor, offset=st[:].offset + off,
                    ap=[list(st[:].ap[0]), [512, 2], [1, 512 - off]])
                etv = bass.AP(
                    tensor=et.tensor, offset=et[:].offset + off,
                    ap=[list(et[:].ap[0]), [512, 2], [1, 512 - off]])
                nc.scalar.activation(etv, stv, EXP)
            j = kt - 4 * qb
            if j >= 0:
                mw = 512 - off
                etm = bass.AP(
                    tensor=et.tensor, offset=et[:].offset + off,
                    ap=[list(et[:].ap[0]), [512, 2], [1, mw]])
                mbc = bass.AP(
                    tensor=mask_sb.tensor, offset=mask_sb[:].offset,
                    ap=[list(mask_sb[:].ap[0]), [0, 2], [1, mw]])
                nc.vector.tensor_mul(etm, etm, mbc)
            nc.tensor.matmul(
                pv[0:65, off:512], v_all[kt][:, vsl0],
                et[:, off:512], start=(kt == 0), stop=(kt == nkt - 1))
            nc.tensor.matmul(
                pv[0:65, 512 + off:1024], v_all[kt][:, vsl1],
                et[:, 512 + off:1024], start=(kt == 0), stop=(kt == nkt - 1))

        def finish_qb(qb, pv):
            qsl = slice(qb * 512, (qb + 1) * 512)
            eng = nc.sync
            # free pv fast: one wide copy for y, one for the dens
            ytu = po_ytu.tile([64, 1024], F16, tag="ytu", name="ytu")
            nc.vector.tensor_copy(ytu[:], pv[0:64, :])
            den = po_den.tile([1, 1024], F32, tag="den", name="den")
            nc.vector.tensor_copy(den[:], pv[64:65, :])
            # reshape [1,1024] -> [128,8] (one sbuf->sbuf DMA) so the
            # reciprocal uses all 128 lanes, then bounce-broadcast fp16
            dd = po_den.tile([128, 8], F32, tag="dd", name="dd")
            eng.dma_start(out=dd[:], in_=den[:])
            nc.vector.reciprocal_approx_fast(dd[:], dd[:])
            ddh = po_den.tile([128, 8], F16, tag="ddh", name="ddh")
            nc.vector.tensor_copy(ddh[:], dd[:])
            dscr2 = po_dram.tile([128, 8], F16, tag="dscr2", name="dscr2")
            eng.dma_start(out=dscr2[:], in_=ddh[:])
            rec = po_rec.tile([64, 1024], F16, tag="recip", name="rec")
            eng.dma_start(out=rec[:], in_=bass.AP(
                tensor=dscr2.tensor, offset=dscr2[:].offset,
                ap=[[0, 64], [1, 1024]]))

            def part_b():
                # deferred so the rec-waiting muls never head-of-line
                # block later DVE work (masks) behind a DMA wait
                nc.vector.tensor_mul(yt[0:64, qsl], ytu[:, 0:512],
                                     rec[:, 0:512])
                ytmp = po_ytu.tile([64, 512], F16, tag="ytmp", name="ytmp")
                nc.vector.tensor_mul(ytmp[:], ytu[:, 512:1024],
                                     rec[:, 512:1024])
                eng.dma_start(out=yt[64:128, qsl], in_=ytmp[:])
            state["fin"] = part_b

        def flush():
            p = state["pend"]
            if p is None:
                return
            state["pend"] = None
            emit_exp_pv(*p)
            st, kt, off, qb, nkt, pv = p
            if kt == nkt - 1:
                finish_qb(qb, pv)

        def run_fin():
            fin = state.pop("fin", None)
            if fin:
                fin()

        def kt_unit(qb, kt, nkt):
            def unit():
                if kt == 0:
                    state["pv"] = pp_pv.tile([128, 1024], F32, tag="pv",
                                             name="pv")
                pv = state["pv"]
                j = kt - 4 * qb
                off = 128 * j if j > 0 else 0
                st = pp_st.tile([128, 1024], F32, tag="st", name="st")
                qsl = slice(qb * 512 + off, (qb + 1) * 512)
                nc.tensor.matmul(st[:, off:512],
                                 kT[0:64, kt * 128:(kt + 1) * 128],
                                 qT[0:64, qsl], start=True, stop=True)
                nc.tensor.matmul(st[:, 512 + off:1024],
                                 kT[64:128, kt * 128:(kt + 1) * 128],
                                 qT[64:128, qsl], start=True, stop=True)
                flush()
                if kt == 2:
                    run_fin()
                state["pend"] = (st, kt, off, qb, nkt, pv)
            return unit

        units = []
        qb_order = (3, 2, 1, 0) if pair == NPAIR - 1 else (0, 1, 2, 3)
        for qb in qb_order:
            nkt = 4 * qb + 4
            for kt in range(nkt):
                units.append(kt_unit(qb, kt, nkt))
        def tail():
            flush()
            run_fin()
        units.append(tail)
        return units

    # ---- fused projection over all 4 pairs ----
    wp_sb = po_wp.tile([128, 8 * 512], F16, tag="wp", name="wp_sb")

    def load_wp():
        # two descriptors: wp_sb[:, (pr*2+cb)*512+j] = wp[pr*128+p, cb*512+j]
        for cb in range(2):
            nc.sync.dma_start(
                out=bass.AP(tensor=wp_sb.tensor,
                            offset=wp_sb[:].offset + cb * 512,
                            ap=[list(wp_sb[:].ap[0]), [1024, 4], [1, 512]]),
                in_=bass.AP(tensor=wp.tensor, offset=wp.offset + cb * 512,
                            ap=[[wp.ap[-2][0], 128],
                                [wp.ap[-2][0] * 128, 4], [1, 512]]))

    def proj_unit(tt, yts):
        def unit():
            ot = po_ot.tile([128, C], F16, tag="ot", name="ot")
            for cb in range(2):
                ps = pp_qk.tile([128, 512], F32, tag="qk", name="pp")
                for pr in range(4):
                    nc.tensor.matmul(
                        ps[:], yts[pr][:, tt * 128:(tt + 1) * 128],
                        wp_sb[:, (pr * 2 + cb) * 512:(pr * 2 + cb + 1) * 512],
                        start=(pr == 0), stop=(pr == 3))
                nc.vector.tensor_copy(ot[:, cb * 512:(cb + 1) * 512], ps[:])
            # dispatch output stores from the ACT hwdge queue -- it is idle
            # during the projection phase and keeps Sync free for the
            # latency-critical denominator bounces
            nc.scalar.dma_start(out=out_d[tt * 128:(tt + 1) * 128, :],
                                in_=ot[:])
        return unit

    def paced_merge(main, side, min_lead=0):
        """Interleave side units evenly into main units; no side unit
        before main index min_lead (so deferred writes they depend on have
        been emitted)."""
        if not side:
            for u in main:
                u()
            return
        n, m = len(main), len(side)
        k = 0
        for i, u in enumerate(main):
            u()
            if i + 1 < max(min_lead, 1):
                continue
            want = (i + 1) * m // n
            while k < want:
                side[k]()
                k += 1
        while k < m:
            side[k]()
            k += 1

    # ---- schedule ----
    st0 = prep_qkv(0)
    load_xt_quarter(0, nc.sync)
    load_wv(nc.scalar)
    load_xt_quarter(1, nc.sync)
    load_xt_quarter(2, nc.scalar)
    load_xt_quarter(3, nc.scalar)
    # quarter-paced qkv(pair0) + v production covering the input DMA ramp;
    # pair-0 attention q-blocks run one quarter behind their inputs
    yts = []
    pair_state = [st0]
    yt0 = po_yt.tile([128, T], F16, tag="yT", name="yt")
    yts.append(yt0)
    attn0 = attn_units(st0, yt0)   # ascending: qb0(4), qb1(8), qb2(12), qb3(16)
    qb_at = [0, 0, 4, 12, 24]
    for q in range(4):
        qkv_unit(st0, q)
        for tt in range(4 * q, 4 * q + 4):
            v_unit(tt)
        for u in attn0[qb_at[q]:qb_at[q + 1]]:
            u()
    for p in range(NPAIR):
        yt = yts[0] if p == 0 else po_yt.tile([128, T], F16, tag="yT",
                                              name="yt")
        if p > 0:
            yts.append(yt)
        side = []
        if p + 1 < NPAIR:
            stn = prep_qkv(p + 1)
            pair_state.append(stn)
            side = [(lambda s=stn, tb=tb: qkv_unit(s, tb)) for tb in range(NTB)]
        if p == 2:
            side.append(load_wp)
        attn = attn0[24:] if p == 0 else attn_units(pair_state[p], yt)
        if p < NPAIR - 1:
            paced_merge(attn, side)
        else:
            # pair 3 runs q-blocks descending; projection for a q-block's
            # four t-tiles unlocks as soon as that q-block finishes.
            qb_sizes = [16, 12, 8, 4]
            qb_tts = [(12, 16), (8, 12), (4, 8), (0, 4)]
            pos = 0
            proj_ready = []
            for i, sz in enumerate(qb_sizes):
                chunk = attn[pos:pos + sz]
                pos += sz
                paced_merge(chunk, proj_ready, min_lead=4)
                proj_ready = [proj_unit(tt, yts)
                              for tt in range(*qb_tts[i])]
            for u in attn[pos:]:
                u()          # trailing flush: finishes qb0
            for u in proj_ready:
                u()

    ctx.close()


_CACHE = {}


def _build():
    if "nc" in _CACHE:
        return _CACHE["nc"]
    nc = bacc.Bacc("TRN2", target_bir_lowering=False, debug=False,
                   enable_asserts=True, num_devices=N_CORES)
    aps = {
        "xt": nc.dram_tensor("xt", [C, T], F16, kind="ExternalInput").ap(),
        "wq": nc.dram_tensor("wq", [C, F], F16, kind="ExternalInput").ap(),
        "wk": nc.dram_tensor("wk", [C, F], F16, kind="ExternalInput").ap(),
        "wva": nc.dram_tensor("wva", [C, VW], F16, kind="ExternalInput").ap(),
        "bq": nc.dram_tensor("bq", [F, 1], F32, kind="ExternalInput").ap(),
        "bk": nc.dram_tensor("bk", [F, 1], F32, kind="ExternalInput").ap(),
        "bva2": nc.dram_tensor("bva2", [1, VW], F32, kind="ExternalInput").ap(),
        "wp": nc.dram_tensor("wp", [F, C], F16, kind="ExternalInput").ap(),
        "mask01": nc.dram_tensor("mask01", [128, 512], F16,
                                 kind="ExternalInput").ap(),
        "out": nc.dram_tensor("out", [T, C], F16,
                              kind="ExternalOutput").ap(),
    }
    with tile.TileContext(nc) as tc:
        _emit(tc, aps)
    nc.compile()
    _CACHE["nc"] = nc
    return nc


def _make_in_maps(x, Wqkv, bqkv, Wproj):
    x = np.asarray(x, dtype=np.float32)
    Wqkv = np.asarray(Wqkv, dtype=np.float32)
    bqkv = np.asarray(bqkv, dtype=np.float32)
    Wproj = np.asarray(Wproj, dtype=np.float32)

    # 0/1 causal mask: visible (1) when tq-within-block >= tk-partition
    p_idx = np.arange(128)[:, None]
    u_idx = np.arange(512)[None, :]
    mask01 = (u_idx >= p_idx).astype(np.float16)

    in_maps = []
    for core in range(N_CORES):
        b, g = divmod(core, 2)
        q0, k0, v0 = 512 * g, C + 512 * g, 2 * C + 512 * g
        wva = np.zeros((C, VW), dtype=np.float32)
        bva = np.zeros((1, VW), dtype=np.float32)
        for h in range(NH):
            src = v0 + D * h
            dst = 65 * h
            # per-head layout [v(64), one]
            wva[:, dst:dst + 64] = Wqkv[:, src:src + 64]
            bva[0, dst:dst + 64] = bqkv[src:src + 64]
            bva[0, dst + 64] = 1.0
        in_maps.append({
            "xt": np.ascontiguousarray(x[b].T).astype(np.float16),
            "wq": np.ascontiguousarray(Wqkv[:, q0:q0 + F]).astype(np.float16),
            "wk": np.ascontiguousarray(Wqkv[:, k0:k0 + F]).astype(np.float16),
            "wva": wva.astype(np.float16),
            "bq": np.ascontiguousarray(bqkv[q0:q0 + F].reshape(F, 1) * 0.125),
            "bk": np.ascontiguousarray(bqkv[k0:k0 + F].reshape(F, 1)),
            "bva2": bva,
            "wp": np.ascontiguousarray(
                Wproj[512 * g:512 * g + F, :]).astype(np.float16),
            "mask01": mask01,
        })
    return in_maps


def run_sharded(x, Wqkv, bqkv, Wproj, bproj, trace=False):
    nc = _build()
    in_maps = _make_in_maps(x, Wqkv, bqkv, Wproj)
    res = run_bass_kernel_spmd(nc, in_maps, core_ids=list(range(N_CORES)),
                               trace=trace)
    bproj = np.asarray(bproj, dtype=np.float32)
    out = np.empty((B, T, C), dtype=np.float32)
    for b in range(B):
        out[b] = (bproj[None, :]
                  + res.results[2 * b]["out"].astype(np.float32)
                  + res.results[2 * b + 1]["out"].astype(np.float32))
    return out, res


def kernel(x, Wqkv, bqkv, Wproj, bproj):
    out, _ = run_sharded(x, Wqkv, bqkv, Wproj, bproj, trace=False)
    return out


# revision 37
# speedup vs baseline: 1.0129x; 1.0129x over previous
"""Causal self-attention (B=4, T=2048, C=1024, H=16, Dh=64) on 8 trn2 NeuronCores.

Sharding: core i <-> (batch b = i//2, head-group g = i%2). Each core computes
8 heads of one batch end-to-end (qkv slice, causal attention, output
projection over its 512 features); the host sums the two head-group partials
per batch and adds bproj. No device collectives.

v2 layout notes:
- x arrives pre-transposed from host (xT [C, T] fp16) -- no PE transposes.
- Scores for the two heads of a pair are issued back-to-back with disjoint
  row-groups (lhsT base partitions 0 / 64), so the PE runs them concurrently
  in separate 64-row strips. Both land in one [128, 1024] PSUM tile (h0|h1),
  which gets a single fused Exp when the diagonal offset is 0.
- Causal mask applied as a post-exp fp16 multiply (cheaper than fp32 add).
- Softmax denominators come from an extra ones column interleaved into Wv
  (pv row 64); reciprocal is taken on [1,512] then broadcast to 64
  partitions via a DRAM bounce in fp16.
- pv PSUM tiles are released immediately via f16 copies so the next q-block's
  PV accumulation doesn't wait on the denominator DMA roundtrip.
- One fused projection over all 4 pairs, fp16 partial output, summed on host.
"""

import numpy as np

import concourse.bass as bass
import concourse.tile as tile
from concourse import bacc, mybir
from concourse.bass_utils import run_bass_kernel_spmd

F32 = mybir.dt.float32
F16 = mybir.dt.float16

N_CORES = 8
B, T, C = 4, 2048, 1024
NH_TOT, D = 16, 64
F = 512            # features per core (8 heads)
NH = 8             # local heads
NPAIR = 4          # head pairs (128 feats each)
CCH = C // 128     # 8 contraction chunks
NTT = T // 128     # 16 t tiles
NTB = T // 512     # 4 t blocks (qkv production)
NQB = T // 512     # 4 q blocks (attention)
VW = NH * (D + 1)  # 520: augmented v width
ADD = mybir.AluOpType.add
MULT = mybir.AluOpType.mult
EXP = mybir.ActivationFunctionType.Exp


def _emit(tc, aps):
    from contextlib import ExitStack
    nc = tc.nc
    xt, wq, wva, bq, bk, wp = (
        aps["xt"], aps["wq"], aps["wva"], aps["bq"], aps["bk"], aps["wp"])
    mask01 = aps["mask01"]
    out_d = aps["out"]

    ctx = ExitStack()
    pp_st = ctx.enter_context(tc.tile_pool(name="ps_st", bufs=2, space="PSUM"))
    pp_pv = ctx.enter_context(tc.tile_pool(name="ps_pv", bufs=1, space="PSUM"))
    pp_qk = ctx.enter_context(tc.tile_pool(name="ps_qk", bufs=2, space="PSUM"))
    po_xt = ctx.enter_context(tc.tile_pool(name="xT", bufs=1))
    po_v = ctx.enter_context(tc.tile_pool(name="v_all", bufs=1))
    po_mask = ctx.enter_context(tc.tile_pool(name="mask", bufs=1))
    po_wv = ctx.enter_context(tc.tile_pool(name="wv", bufs=1))
    po_wqk = ctx.enter_context(tc.tile_pool(name="wqk", bufs=2))
    po_qkt = ctx.enter_context(tc.tile_pool(name="qkT", bufs=2))
    po_bias = ctx.enter_context(tc.tile_pool(name="bias", bufs=1))
    po_exp = ctx.enter_context(tc.tile_pool(name="expT", bufs=3))
    po_yt = ctx.enter_context(tc.tile_pool(name="yT", bufs=4))
    po_ytu = ctx.enter_context(tc.tile_pool(name="ytu", bufs=3))
    po_den = ctx.enter_context(tc.tile_pool(name="den", bufs=4))
    po_rec = ctx.enter_context(tc.tile_pool(name="recip", bufs=4))
    po_wp = ctx.enter_context(tc.tile_pool(name="wp", bufs=1))
    po_ot = ctx.enter_context(tc.tile_pool(name="ot", bufs=3))
    po_dram = ctx.enter_context(tc.tile_pool(name="dram_scr", bufs=4,
                                             space="DRAM"))

    mask_sb = po_mask.tile([128, 512], F16, tag="mask")
    nc.sync.dma_start(out=mask_sb[:], in_=mask01[:])
    # bva broadcast to all 128 partitions straight from DRAM
    bva_bc = po_bias.tile([128, VW], F32, tag="bva_bc")
    bva2 = aps["bva2"]
    nc.sync.dma_start(out=bva_bc[:], in_=bass.AP(
        tensor=bva2.tensor, offset=bva2.offset,
        ap=[[0, 128]] + [list(a) for a in bva2.ap[1:]]))

    # xT: one [128, 8*T] tile (chunk-major), so a whole 2MB half loads as
    # a single fat DMA descriptor.
    xTt = po_xt.tile([128, CCH * T], F16, tag="xT", name="xTt")

    def xT(c):
        return xTt[:, c * T:(c + 1) * T]

    def load_xt_quarter(q, eng):
        # host-packed quarter: DRAM rows are contiguous 8KB; SBUF side
        # interleaves the 8 chunks into xTt
        eng.dma_start(
            out=bass.AP(tensor=xTt.tensor, offset=xTt[:].offset + q * 512,
                        ap=[list(xTt[:].ap[0]), [T, CCH], [1, 512]]),
            in_=bass.AP(tensor=xt.tensor, offset=xt.offset + q * 128 * 4096,
                        ap=[[4096, 128], [512, CCH], [1, 512]]))

    # ---- v production (augmented with ones columns, all 8 heads) ----
    v_all = [po_v.tile([128, VW], F16, tag=f"v{tt}", name=f"v{tt}")
             for tt in range(NTT)]
    wv_sb = po_wv.tile([128, CCH * VW], F16, tag="wv", name="wv_sb")

    def load_wv(eng):
        eng.dma_start(out=wv_sb[:], in_=wva[0:128, :])

    def v_unit(tt):
        for half in range(2):
            cs = slice(half * 260, half * 260 + 260)
            ps = pp_qk.tile([128, 260], F32, tag="qk")
            for c in range(CCH):
                nc.tensor.matmul(
                    ps[:], xT(c)[:, tt * 128:(tt + 1) * 128],
                    wv_sb[:, c * VW + half * 260:c * VW + half * 260 + 260],
                    start=(c == 0), stop=(c == CCH - 1))
            nc.vector.tensor_add(v_all[tt][:, cs], ps[:], bva_bc[:, cs])

    # ---- qkv per pair ----
    def prep_qkv(pair, eng=None):
        eng = eng or nc.sync
        psl = pair * 128
        wqk = po_wqk.tile([128, CCH * 256], F16, tag="wqk", name="wqk")
        # host-packed: one contiguous [128, 2048] descriptor per pair
        eng.dma_start(out=wqk[:],
                      in_=wq[pair * 128:(pair + 1) * 128, :])
        bq_sb = po_bias.tile([128, 1], F32, tag=f"bq{pair}", name=f"bq{pair}")
        eng.dma_start(out=bq_sb[:], in_=bq[psl:psl + 128, :])
        bk_sb = po_bias.tile([128, 1], F32, tag=f"bk{pair}", name=f"bk{pair}")
        eng.dma_start(out=bk_sb[:], in_=bk[psl:psl + 128, :])
        qT = po_qkt.tile([128, T], F16, tag="qT", name="qT")
        kT = po_qkt.tile([128, T], F16, tag="kT", name="kT")
        return dict(wqk=wqk, bq=bq_sb, bk=bk_sb, qT=qT, kT=kT, pair=pair)

    def qkv_unit(st8, tb):
        tsl = slice(tb * 512, (tb + 1) * 512)
        wqk = st8["wqk"]
        psq = pp_qk.tile([128, 512], F32, tag="qk", name="psq")
        for c in range(CCH):
            nc.tensor.matmul(psq[:], wqk[:, c * 256:c * 256 + 128],
                             xT(c)[:, tsl],
                             start=(c == 0), stop=(c == CCH - 1))
        # psum*1/sqrt(D) + bq/sqrt(D)   (bq pre-scaled on host)
        nc.vector.tensor_scalar(
            out=st8["qT"][:, tsl], in0=psq[:], scalar1=0.125,
            scalar2=st8["bq"][:], op0=MULT, op1=ADD)
        psk = pp_qk.tile([128, 512], F32, tag="qk", name="psk")
        for c in range(CCH):
            nc.tensor.matmul(psk[:], wqk[:, c * 256 + 128:c * 256 + 256],
                             xT(c)[:, tsl],
                             start=(c == 0), stop=(c == CCH - 1))
        nc.vector.tensor_scalar(
            out=st8["kT"][:, tsl], in0=psk[:], scalar1=st8["bk"][:],
            scalar2=None, op0=ADD)

    # ---- attention: unit = one ktile-event of one q-block ----
    # Scores for both heads go into one [128, 1024] PSUM tile (h0 | h1),
    # issued adjacently with row-disjoint lhsT slices so the PE overlaps
    # them. exp(kt-1) is emitted between scores(kt) and pv(kt-1) so the
    # scalar engine runs one ktile ahead of the PV consumer.
    def attn_units(st8, yt):
        qT, kT = st8["qT"], st8["kT"]
        pair = st8["pair"]
        vsl0 = slice((pair * 2) * 65, (pair * 2) * 65 + 65)
        vsl1 = slice((pair * 2 + 1) * 65, (pair * 2 + 1) * 65 + 65)
        state = {"pend": None, "pv": None}

        def emit_exp_pv(st, kt, off, qb, nkt, pv):
            et = po_exp.tile([128, 1024], F16, tag="expT", name="et")
            if off == 0:
                nc.scalar.activation(et[:, 0:1024], st[:, 0:1024], EXP)
            else:
                # one activation over both heads' valid slices via a 3D AP
                stv = bass.AP(
                    tensor=st.tensor, offset=st[:].offset + off,
                    ap=[list(st[:].ap[0]), [512, 2], [1, 512 - off]])
                etv = bass.AP(
                    tensor=et.tensor, offset=et[:].offset + off,
                    ap=[list(et[:].ap[0]), [512, 2], [1, 512 - off]])
                nc.scalar.activation(etv, stv, EXP)
            j = kt - 4 * qb
            if j >= 0:
                mw = 512 - off
                etm = bass.AP(
                    tensor=et.tensor, offset=et[:].offset + off,
                    ap=[list(et[:].ap[0]), [512, 2], [1, mw]])
                mbc = bass.AP(
                    tensor=mask_sb.tensor, offset=mask_sb[:].offset,
                    ap=[list(mask_sb[:].ap[0]), [0, 2], [1, mw]])
                nc.vector.tensor_mul(etm, etm, mbc)
            nc.tensor.matmul(
                pv[0:65, off:512], v_all[kt][:, vsl0],
                et[:, off:512], start=(kt == 0), stop=(kt == nkt - 1))
            nc.tensor.matmul(
                pv[0:65, 512 + off:1024], v_all[kt][:, vsl1],
                et[:, 512 + off:1024], start=(kt == 0), stop=(kt == nkt - 1))

        def finish_qb(qb, pv):
            qsl = slice(qb * 512, (qb + 1) * 512)
            eng = (nc.scalar if (pair == NPAIR - 1 and qb == 0)
                   else nc.sync)
            # free pv fast: one wide copy for y, one for the dens
            ytu = po_ytu.tile([64, 1024], F16, tag="ytu", name="ytu")
            nc.vector.tensor_copy(ytu[:], pv[0:64, :])
            den = po_den.tile([1, 1024], F32, tag="den", name="den")
            nc.vector.tensor_copy(den[:], pv[64:65, :])
            # reshape [1,1024] -> [128,8] (one sbuf->sbuf DMA) so the
            # reciprocal uses all 128 lanes, then bounce-broadcast fp16
            dd = po_den.tile([128, 8], F32, tag="dd", name="dd")
            eng.dma_start(out=dd[:], in_=den[:])
            nc.vector.reciprocal_approx_fast(dd[:], dd[:])
            ddh = po_den.tile([128, 8], F16, tag="ddh", name="ddh")
            nc.vector.tensor_copy(ddh[:], dd[:])
            dscr2 = po_dram.tile([128, 8], F16, tag="dscr2", name="dscr2")
            eng.dma_start(out=dscr2[:], in_=ddh[:])
            rec = po_rec.tile([64, 1024], F16, tag="recip", name="rec")
            eng.dma_start(out=rec[:], in_=bass.AP(
                tensor=dscr2.tensor, offset=dscr2[:].offset,
                ap=[[0, 64], [1, 1024]]))

            def part_b():
                # deferred so the rec-waiting muls never head-of-line
                # block later DVE work (masks) behind a DMA wait
                nc.vector.tensor_mul(yt[0:64, qsl], ytu[:, 0:512],
                                     rec[:, 0:512])
                ytmp = po_ytu.tile([64, 512], F16, tag="ytmp", name="ytmp")
                nc.vector.tensor_mul(ytmp[:], ytu[:, 512:1024],
                                     rec[:, 512:1024])
                eng.dma_start(out=yt[64:128, qsl], in_=ytmp[:])
            state["fin"] = part_b

        def flush():
            p = state["pend"]
            if p is None:
                return
            state["pend"] = None
            emit_exp_pv(*p)
            st, kt, off, qb, nkt, pv = p
            if kt == nkt - 1:
                finish_qb(qb, pv)

        def run_fin():
            fin = state.pop("fin", None)
            if fin:
                fin()

        def kt_unit(qb, kt, nkt):
            def unit():
                if kt == 0:
                    state["pv"] = pp_pv.tile([128, 1024], F32, tag="pv",
                                             name="pv")
                pv = state["pv"]
                j = kt - 4 * qb
                off = 128 * j if j > 0 else 0
                st = pp_st.tile([128, 1024], F32, tag="st", name="st")
                qsl = slice(qb * 512 + off, (qb + 1) * 512)
                nc.tensor.matmul(st[:, off:512],
                                 kT[0:64, kt * 128:(kt + 1) * 128],
                                 qT[0:64, qsl], start=True, stop=True)
                nc.tensor.matmul(st[:, 512 + off:1024],
                                 kT[64:128, kt * 128:(kt + 1) * 128],
                                 qT[64:128, qsl], start=True, stop=True)
                flush()
                if kt == 2:
                    run_fin()
                state["pend"] = (st, kt, off, qb, nkt, pv)
            return unit

        units = []
        qb_order = (3, 2, 1, 0) if pair == NPAIR - 1 else (0, 1, 2, 3)
        for qb in qb_order:
            nkt = 4 * qb + 4
            for kt in range(nkt):
                units.append(kt_unit(qb, kt, nkt))
        def tail():
            flush()
            run_fin()
        units.append(tail)
        return units

    # ---- fused projection over all 4 pairs ----
    wp_sb = po_wp.tile([128, 8 * 512], F16, tag="wp", name="wp_sb")

    def load_wp():
        nc.sync.dma_start(out=wp_sb[:], in_=wp[0:128, :])

    def proj_unit(tt, yts):
        def unit():
            ot = po_ot.tile([128, C], F16, tag="ot", name="ot")
            for cb in range(2):
                ps = pp_qk.tile([128, 512], F32, tag="qk", name="pp")
                for pr in range(4):
                    nc.tensor.matmul(
                        ps[:], yts[pr][:, tt * 128:(tt + 1) * 128],
                        wp_sb[:, (pr * 2 + cb) * 512:(pr * 2 + cb + 1) * 512],
                        start=(pr == 0), stop=(pr == 3))
                nc.vector.tensor_copy(ot[:, cb * 512:(cb + 1) * 512], ps[:])
            # dispatch output stores from the ACT hwdge queue -- it is idle
            # during the projection phase and keeps Sync free for the
            # latency-critical denominator bounces
            nc.scalar.dma_start(out=out_d[tt * 128:(tt + 1) * 128, :],
                                in_=ot[:])
        return unit

    def paced_merge(main, side, min_lead=0):
        """Interleave side units evenly into main units; no side unit
        before main index min_lead (so deferred writes they depend on have
        been emitted)."""
        if not side:
            for u in main:
                u()
            return
        n, m = len(main), len(side)
        k = 0
        for i, u in enumerate(main):
            u()
            if i + 1 < max(min_lead, 1):
                continue
            want = (i + 1) * m // n
            while k < want:
                side[k]()
                k += 1
        while k < m:
            side[k]()
            k += 1

    # ---- schedule ----
    st0 = prep_qkv(0)
    load_xt_quarter(0, nc.sync)
    load_wv(nc.scalar)
    load_xt_quarter(1, nc.sync)
    load_xt_quarter(2, nc.scalar)
    load_xt_quarter(3, nc.scalar)
    # quarter-paced qkv(pair0) + v production covering the input DMA ramp;
    # pair-0 attention q-blocks run one quarter behind their inputs
    yts = []
    pair_state = [st0]
    yt0 = po_yt.tile([128, T], F16, tag="yT", name="yt")
    yts.append(yt0)
    attn0 = attn_units(st0, yt0)   # ascending: qb0(4), qb1(8), qb2(12), qb3(16)
    qb_at = [0, 0, 4, 12, 24]
    for q in range(4):
        qkv_unit(st0, q)
        for tt in range(4 * q, 4 * q + 4):
            v_unit(tt)
        for u in attn0[qb_at[q]:qb_at[q + 1]]:
            u()
    for p in range(NPAIR):
        yt = yts[0] if p == 0 else po_yt.tile([128, T], F16, tag="yT",
                                              name="yt")
        if p > 0:
            yts.append(yt)
        side = []
        if p + 1 < NPAIR:
            stn = prep_qkv(p + 1)
            pair_state.append(stn)
            side = [(lambda s=stn, tb=tb: qkv_unit(s, tb)) for tb in range(NTB)]
        if p == 2:
            side.append(load_wp)
        attn = attn0[24:] if p == 0 else attn_units(pair_state[p], yt)
        if p < NPAIR - 1:
            paced_merge(attn, side)
        else:
            # pair 3 runs q-blocks descending; projection for a q-block's
            # four t-tiles unlocks as soon as that q-block finishes.
            qb_sizes = [16, 12, 8, 4]
            qb_tts = [(12, 16), (8, 12), (4, 8), (0, 4)]
            pos = 0
            proj_ready = []
            for i, sz in enumerate(qb_sizes):
                chunk = attn[pos:pos + sz]
                pos += sz
                paced_merge(chunk, proj_ready, min_lead=4)
                proj_ready = [proj_unit(tt, yts)
                              for tt in range(*qb_tts[i])]
            for u in attn[pos:]:
                u()          # trailing flush: finishes qb0
            for u in proj_ready:
                u()

    ctx.close()


_CACHE = {}


def _build():
    if "nc" in _CACHE:
        return _CACHE["nc"]
    nc = bacc.Bacc("TRN2", target_bir_lowering=False, debug=False,
                   enable_asserts=True, num_devices=N_CORES)
    aps = {
        "xt": nc.dram_tensor("xt", [512, 4096], F16,
                             kind="ExternalInput").ap(),
        "wq": nc.dram_tensor("wq", [512, CCH * 256], F16,
                             kind="ExternalInput").ap(),
        "wva": nc.dram_tensor("wva", [128, CCH * VW], F16,
                              kind="ExternalInput").ap(),
        "bq": nc.dram_tensor("bq", [F, 1], F32, kind="ExternalInput").ap(),
        "bk": nc.dram_tensor("bk", [F, 1], F32, kind="ExternalInput").ap(),
        "bva2": nc.dram_tensor("bva2", [1, VW], F32, kind="ExternalInput").ap(),
        "wp": nc.dram_tensor("wp", [128, 8 * 512], F16,
                             kind="ExternalInput").ap(),
        "mask01": nc.dram_tensor("mask01", [128, 512], F16,
                                 kind="ExternalInput").ap(),
        "out": nc.dram_tensor("out", [T, C], F16,
                              kind="ExternalOutput").ap(),
    }
    with tile.TileContext(nc) as tc:
        _emit(tc, aps)
    nc.compile()
    _CACHE["nc"] = nc
    return nc


def _make_in_maps(x, Wqkv, bqkv, Wproj):
    x = np.asarray(x, dtype=np.float32)
    Wqkv = np.asarray(Wqkv, dtype=np.float32)
    bqkv = np.asarray(bqkv, dtype=np.float32)
    Wproj = np.asarray(Wproj, dtype=np.float32)

    # 0/1 causal mask: visible (1) when tq-within-block >= tk-partition
    p_idx = np.arange(128)[:, None]
    u_idx = np.arange(512)[None, :]
    mask01 = (u_idx >= p_idx).astype(np.float16)

    in_maps = []
    for core in range(N_CORES):
        b, g = divmod(core, 2)
        q0, k0, v0 = 512 * g, C + 512 * g, 2 * C + 512 * g
        wva = np.zeros((C, VW), dtype=np.float32)
        bva = np.zeros((1, VW), dtype=np.float32)
        for h in range(NH):
            src = v0 + D * h
            dst = 65 * h
            # per-head layout [v(64), one]
            wva[:, dst:dst + 64] = Wqkv[:, src:src + 64]
            bva[0, dst:dst + 64] = bqkv[src:src + 64]
            bva[0, dst + 64] = 1.0
        # xq[q*128+p, c*512+j] = x[b][q*512+j, c*128+p]
        xq = (x[b].astype(np.float16).T          # [C, T]
              .reshape(CCH, 128, 4, 512)         # c, p, q, j
              .transpose(2, 1, 0, 3)             # q, p, c, j
              .reshape(512, 4096))
        # wqk[pair*128+p, c*256+j] = (wq|wk)[c*128+p, pair*128+j']
        wq_ = Wqkv[:, q0:q0 + F].astype(np.float16).reshape(CCH, 128, 4, 128)
        wk_ = Wqkv[:, k0:k0 + F].astype(np.float16).reshape(CCH, 128, 4, 128)
        wqk = np.concatenate([wq_, wk_], axis=3)  # c, p, pair, 256
        wqk = wqk.transpose(2, 1, 0, 3).reshape(512, CCH * 256)
        # wvp[p, c*520+j] = wva[c*128+p, j]
        wvp = (wva.astype(np.float16).reshape(CCH, 128, VW)
               .transpose(1, 0, 2).reshape(128, CCH * VW))
        # wpp[p, (pr*2+cb)*512+j] = Wproj[512g + pr*128+p, cb*512+j]
        wpp = (Wproj[512 * g:512 * g + F, :].astype(np.float16)
               .reshape(4, 128, 2, 512).transpose(1, 0, 2, 3)
               .reshape(128, 8 * 512))
        in_maps.append({
            "xt": np.ascontiguousarray(xq),
            "wq": np.ascontiguousarray(wqk),
            "wva": np.ascontiguousarray(wvp),
            "bq": np.ascontiguousarray(bqkv[q0:q0 + F].reshape(F, 1) * 0.125),
            "bk": np.ascontiguousarray(bqkv[k0:k0 + F].reshape(F, 1)),
            "bva2": bva,
            "wp": np.ascontiguousarray(wpp),
            "mask01": mask01,
        })
    return in_maps


def run_sharded(x, Wqkv, bqkv, Wproj, bproj, trace=False):
    nc = _build()
    in_maps = _make_in_maps(x, Wqkv, bqkv, Wproj)
    res = run_bass_kernel_spmd(nc, in_maps, core_ids=list(range(N_CORES)),
                               trace=trace)
    bproj = np.asarray(bproj, dtype=np.float32)
    out = np.empty((B, T, C), dtype=np.float32)
    for b in range(B):
        out[b] = (bproj[None, :]
                  + res.results[2 * b]["out"].astype(np.float32)
                  + res.results[2 * b + 1]["out"].astype(np.float32))
    return out, res


def kernel(x, Wqkv, bqkv, Wproj, bproj):
    out, _ = run_sharded(x, Wqkv, bqkv, Wproj, bproj, trace=False)
    return out


# revision 38
# speedup vs baseline: 1.0580x; 1.0446x over previous
"""Causal self-attention (B=4, T=2048, C=1024, H=16, Dh=64) on 8 trn2 NeuronCores.

Sharding: core i <-> (batch b = i//2, head-group g = i%2). Each core computes
8 heads of one batch end-to-end (qkv slice, causal attention, output
projection over its 512 features); the host sums the two head-group partials
per batch and adds bproj. No device collectives.

v2 layout notes:
- x arrives pre-transposed from host (xT [C, T] fp16) -- no PE transposes.
- Scores for the two heads of a pair are issued back-to-back with disjoint
  row-groups (lhsT base partitions 0 / 64), so the PE runs them concurrently
  in separate 64-row strips. Both land in one [128, 1024] PSUM tile (h0|h1),
  which gets a single fused Exp when the diagonal offset is 0.
- Causal mask applied as a post-exp fp16 multiply (cheaper than fp32 add).
- Softmax denominators come from an extra ones column interleaved into Wv
  (pv row 64); reciprocal is taken on [1,512] then broadcast to 64
  partitions via a DRAM bounce in fp16.
- pv PSUM tiles are released immediately via f16 copies so the next q-block's
  PV accumulation doesn't wait on the denominator DMA roundtrip.
- One fused projection over all 4 pairs, fp16 partial output, summed on host.
"""

import numpy as np

import concourse.bass as bass
import concourse.tile as tile
from concourse import bacc, mybir
from concourse.bass_utils import run_bass_kernel_spmd

F32 = mybir.dt.float32
F16 = mybir.dt.float16

N_CORES = 8
B, T, C = 4, 2048, 1024
NH_TOT, D = 16, 64
F = 512            # features per core (8 heads)
NH = 8             # local heads
NPAIR = 4          # head pairs (128 feats each)
CCH = C // 128     # 8 contraction chunks
NTT = T // 128     # 16 t tiles
NTB = T // 512     # 4 t blocks (qkv production)
NQB = T // 512     # 4 q blocks (attention)
VW = NH * (D + 1)  # 520: augmented v width
ADD = mybir.AluOpType.add
MULT = mybir.AluOpType.mult
EXP = mybir.ActivationFunctionType.Exp


def _emit(tc, aps):
    from contextlib import ExitStack
    nc = tc.nc
    xt, wq, wva, bq, bk, wp = (
        aps["xt"], aps["wq"], aps["wva"], aps["bq"], aps["bk"], aps["wp"])
    mask01 = aps["mask01"]
    out_d = aps["out"]

    ctx = ExitStack()
    pp_st = ctx.enter_context(tc.tile_pool(name="ps_st", bufs=2, space="PSUM"))
    pp_pv = ctx.enter_context(tc.tile_pool(name="ps_pv", bufs=1, space="PSUM"))
    pp_qk = ctx.enter_context(tc.tile_pool(name="ps_qk", bufs=2, space="PSUM"))
    po_xt = ctx.enter_context(tc.tile_pool(name="xT", bufs=1))
    po_v = ctx.enter_context(tc.tile_pool(name="v_all", bufs=1))
    po_mask = ctx.enter_context(tc.tile_pool(name="mask", bufs=1))
    po_wv = ctx.enter_context(tc.tile_pool(name="wv", bufs=1))
    po_wqk = ctx.enter_context(tc.tile_pool(name="wqk", bufs=2))
    po_qkt = ctx.enter_context(tc.tile_pool(name="qkT", bufs=2))
    po_bias = ctx.enter_context(tc.tile_pool(name="bias", bufs=1))
    po_exp = ctx.enter_context(tc.tile_pool(name="expT", bufs=3))
    po_yt = ctx.enter_context(tc.tile_pool(name="yT", bufs=4))
    po_ytu = ctx.enter_context(tc.tile_pool(name="ytu", bufs=3))
    po_den = ctx.enter_context(tc.tile_pool(name="den", bufs=4))
    po_rec = ctx.enter_context(tc.tile_pool(name="recip", bufs=4))
    po_wp = ctx.enter_context(tc.tile_pool(name="wp", bufs=1))
    po_ot = ctx.enter_context(tc.tile_pool(name="ot", bufs=3))
    po_dram = ctx.enter_context(tc.tile_pool(name="dram_scr", bufs=4,
                                             space="DRAM"))

    mask_sb = po_mask.tile([128, 512], F16, tag="mask")
    nc.sync.dma_start(out=mask_sb[:], in_=mask01[:])
    # bva broadcast to all 128 partitions straight from DRAM
    bva_bc = po_bias.tile([128, VW], F32, tag="bva_bc")
    bva2 = aps["bva2"]
    nc.sync.dma_start(out=bva_bc[:], in_=bass.AP(
        tensor=bva2.tensor, offset=bva2.offset,
        ap=[[0, 128]] + [list(a) for a in bva2.ap[1:]]))

    # xT: one [128, 8*T] tile (chunk-major), so a whole 2MB half loads as
    # a single fat DMA descriptor.
    xTt = po_xt.tile([128, CCH * T], F16, tag="xT", name="xTt")

    def xT(c):
        return xTt[:, c * T:(c + 1) * T]

    def load_xt_quarter(q, eng):
        # host-packed quarter: DRAM rows are contiguous 8KB; SBUF side
        # interleaves the 8 chunks into xTt
        eng.dma_start(
            out=bass.AP(tensor=xTt.tensor, offset=xTt[:].offset + q * 512,
                        ap=[list(xTt[:].ap[0]), [T, CCH], [1, 512]]),
            in_=bass.AP(tensor=xt.tensor, offset=xt.offset + q * 128 * 4096,
                        ap=[[4096, 128], [512, CCH], [1, 512]]))

    # ---- v production (augmented with ones columns, all 8 heads) ----
    v_all = [po_v.tile([128, VW], F16, tag=f"v{tt}", name=f"v{tt}")
             for tt in range(NTT)]
    wv_sb = po_wv.tile([128, CCH * VW], F16, tag="wv", name="wv_sb")

    def load_wv(eng):
        eng.dma_start(out=wv_sb[:], in_=wva[0:128, :])

    def v_unit(tt):
        for half in range(2):
            cs = slice(half * 260, half * 260 + 260)
            ps = pp_qk.tile([128, 260], F32, tag="qk")
            for c in range(CCH):
                nc.tensor.matmul(
                    ps[:], xT(c)[:, tt * 128:(tt + 1) * 128],
                    wv_sb[:, c * VW + half * 260:c * VW + half * 260 + 260],
                    start=(c == 0), stop=(c == CCH - 1))
            nc.vector.tensor_add(v_all[tt][:, cs], ps[:], bva_bc[:, cs])

    # ---- qkv per pair ----
    def prep_qkv(pair, eng=None):
        eng = eng or nc.sync
        psl = pair * 128
        wqk = po_wqk.tile([128, CCH * 256], F16, tag="wqk", name="wqk")
        # host-packed: one contiguous [128, 2048] descriptor per pair
        eng.dma_start(out=wqk[:],
                      in_=wq[pair * 128:(pair + 1) * 128, :])
        bq_sb = po_bias.tile([128, 1], F32, tag=f"bq{pair}", name=f"bq{pair}")
        eng.dma_start(out=bq_sb[:], in_=bq[psl:psl + 128, :])
        bk_sb = po_bias.tile([128, 1], F32, tag=f"bk{pair}", name=f"bk{pair}")
        eng.dma_start(out=bk_sb[:], in_=bk[psl:psl + 128, :])
        qT = po_qkt.tile([128, T], F16, tag="qT", name="qT")
        kT = po_qkt.tile([128, T], F16, tag="kT", name="kT")
        return dict(wqk=wqk, bq=bq_sb, bk=bk_sb, qT=qT, kT=kT, pair=pair)

    def qkv_unit(st8, tb):
        tsl = slice(tb * 512, (tb + 1) * 512)
        wqk = st8["wqk"]
        psq = pp_qk.tile([128, 512], F32, tag="qk", name="psq")
        for c in range(CCH):
            nc.tensor.matmul(psq[:], wqk[:, c * 256:c * 256 + 128],
                             xT(c)[:, tsl],
                             start=(c == 0), stop=(c == CCH - 1))
        # psum*1/sqrt(D) + bq/sqrt(D)   (bq pre-scaled on host)
        nc.vector.tensor_scalar(
            out=st8["qT"][:, tsl], in0=psq[:], scalar1=0.125,
            scalar2=st8["bq"][:], op0=MULT, op1=ADD)
        psk = pp_qk.tile([128, 512], F32, tag="qk", name="psk")
        for c in range(CCH):
            nc.tensor.matmul(psk[:], wqk[:, c * 256 + 128:c * 256 + 256],
                             xT(c)[:, tsl],
                             start=(c == 0), stop=(c == CCH - 1))
        nc.vector.tensor_scalar(
            out=st8["kT"][:, tsl], in0=psk[:], scalar1=st8["bk"][:],
            scalar2=None, op0=ADD)

    # ---- attention: unit = one ktile-event of one q-block ----
    # Scores for both heads go into one [128, 1024] PSUM tile (h0 | h1),
    # issued adjacently with row-disjoint lhsT slices so the PE overlaps
    # them. exp(kt-1) is emitted between scores(kt) and pv(kt-1) so the
    # scalar engine runs one ktile ahead of the PV consumer.
    def attn_units(st8, yt):
        qT, kT = st8["qT"], st8["kT"]
        pair = st8["pair"]
        vsl0 = slice((pair * 2) * 65, (pair * 2) * 65 + 65)
        vsl1 = slice((pair * 2 + 1) * 65, (pair * 2 + 1) * 65 + 65)
        state = {"pend": None, "pv": None}

        def emit_exp_pv(st, kt, off, qb, nkt, pv):
            et = po_exp.tile([128, 1024], F16, tag="expT", name="et")
            if off == 0:
                nc.scalar.activation(et[:, 0:1024], st[:, 0:1024], EXP)
            else:
                # one activation over both heads' valid slices via a 3D AP
                stv = bass.AP(
                    tensor=st.tensor, offset=st[:].offset + off,
                    ap=[list(st[:].ap[0]), [512, 2], [1, 512 - off]])
                etv = bass.AP(
                    tensor=et.tensor, offset=et[:].offset + off,
                    ap=[list(et[:].ap[0]), [512, 2], [1, 512 - off]])
                nc.scalar.activation(etv, stv, EXP)
            j = kt - 4 * qb
            if j >= 0:
                mw = 512 - off
                etm = bass.AP(
                    tensor=et.tensor, offset=et[:].offset + off,
                    ap=[list(et[:].ap[0]), [512, 2], [1, mw]])
                mbc = bass.AP(
                    tensor=mask_sb.tensor, offset=mask_sb[:].offset,
                    ap=[list(mask_sb[:].ap[0]), [0, 2], [1, mw]])
                nc.vector.tensor_mul(etm, etm, mbc)
            nc.tensor.matmul(
                pv[0:65, off:512], v_all[kt][:, vsl0],
                et[:, off:512], start=(kt == 0), stop=(kt == nkt - 1))
            nc.tensor.matmul(
                pv[0:65, 512 + off:1024], v_all[kt][:, vsl1],
                et[:, 512 + off:1024], start=(kt == 0), stop=(kt == nkt - 1))

        def finish_qb(qb, pv):
            qsl = slice(qb * 512, (qb + 1) * 512)
            last = pair == NPAIR - 1 and qb == 0
            tailish = pair == NPAIR - 1 and qb <= 1
            eng = nc.scalar if last else nc.sync
            # dens first -- they head the longest latency chain; for the
            # final q-blocks copy them on ScalarE (idle then, off the DVE
            # queue). The very last q-block keeps y in PSUM (no ytu copy:
            # nothing needs the pv banks afterwards).
            den = po_den.tile([1, 1024], F32, tag="den", name="den")
            if tailish:
                nc.scalar.copy(den[:], pv[64:65, :])
            else:
                nc.vector.tensor_copy(den[:], pv[64:65, :])
            if last:
                ytu = pv
            else:
                ytu = po_ytu.tile([64, 1024], F16, tag="ytu", name="ytu")
                nc.vector.tensor_copy(ytu[:], pv[0:64, :])
            # reshape [1,1024] -> [128,8] (one sbuf->sbuf DMA) so the
            # reciprocal uses all 128 lanes, then bounce-broadcast fp16
            dd = po_den.tile([128, 8], F32, tag="dd", name="dd")
            eng.dma_start(out=dd[:], in_=den[:])
            nc.vector.reciprocal_approx_fast(dd[:], dd[:])
            ddh = po_den.tile([128, 8], F16, tag="ddh", name="ddh")
            nc.vector.tensor_copy(ddh[:], dd[:])
            dscr2 = po_dram.tile([128, 8], F16, tag="dscr2", name="dscr2")
            eng.dma_start(out=dscr2[:], in_=ddh[:])
            rec = po_rec.tile([64, 1024], F16, tag="recip", name="rec")
            eng.dma_start(out=rec[:], in_=bass.AP(
                tensor=dscr2.tensor, offset=dscr2[:].offset,
                ap=[[0, 64], [1, 1024]]))

            def part_b():
                # deferred so the rec-waiting muls never head-of-line
                # block later DVE work (masks) behind a DMA wait
                nc.vector.tensor_mul(yt[0:64, qsl], ytu[0:64, 0:512],
                                     rec[:, 0:512])
                ytmp = po_ytu.tile([64, 512], F16, tag="ytmp", name="ytmp")
                nc.vector.tensor_mul(ytmp[:], ytu[0:64, 512:1024],
                                     rec[:, 512:1024])
                eng.dma_start(out=yt[64:128, qsl], in_=ytmp[:])
            state["fin"] = part_b

        def flush():
            p = state["pend"]
            if p is None:
                return
            state["pend"] = None
            emit_exp_pv(*p)
            st, kt, off, qb, nkt, pv = p
            if kt == nkt - 1:
                finish_qb(qb, pv)

        def run_fin():
            fin = state.pop("fin", None)
            if fin:
                fin()

        def kt_unit(qb, kt, nkt):
            def unit():
                if kt == 0:
                    state["pv"] = pp_pv.tile([128, 1024], F32, tag="pv",
                                             name="pv")
                pv = state["pv"]
                j = kt - 4 * qb
                off = 128 * j if j > 0 else 0
                st = pp_st.tile([128, 1024], F32, tag="st", name="st")
                qsl = slice(qb * 512 + off, (qb + 1) * 512)
                nc.tensor.matmul(st[:, off:512],
                                 kT[0:64, kt * 128:(kt + 1) * 128],
                                 qT[0:64, qsl], start=True, stop=True)
                nc.tensor.matmul(st[:, 512 + off:1024],
                                 kT[64:128, kt * 128:(kt + 1) * 128],
                                 qT[64:128, qsl], start=True, stop=True)
                flush()
                if kt == 2:
                    run_fin()
                state["pend"] = (st, kt, off, qb, nkt, pv)
            return unit

        units = []
        qb_order = (3, 2, 1, 0) if pair == NPAIR - 1 else (0, 1, 2, 3)
        for qb in qb_order:
            nkt = 4 * qb + 4
            for kt in range(nkt):
                units.append(kt_unit(qb, kt, nkt))
        def tail():
            flush()
            run_fin()
        units.append(tail)
        return units

    # ---- fused projection over all 4 pairs ----
    wp_sb = po_wp.tile([128, 8 * 512], F16, tag="wp", name="wp_sb")

    def load_wp():
        nc.sync.dma_start(out=wp_sb[:], in_=wp[0:128, :])

    def proj_unit(tt, yts):
        def unit():
            ot = po_ot.tile([128, C], F16, tag="ot", name="ot")
            for cb in range(2):
                ps = pp_qk.tile([128, 512], F32, tag="qk", name="pp")
                for pr in range(4):
                    nc.tensor.matmul(
                        ps[:], yts[pr][:, tt * 128:(tt + 1) * 128],
                        wp_sb[:, (pr * 2 + cb) * 512:(pr * 2 + cb + 1) * 512],
                        start=(pr == 0), stop=(pr == 3))
                nc.vector.tensor_copy(ot[:, cb * 512:(cb + 1) * 512], ps[:])
            # alternate output stores across both hwdge queues
            oeng = nc.scalar if tt % 2 else nc.sync
            oeng.dma_start(out=out_d[tt * 128:(tt + 1) * 128, :], in_=ot[:])
        return unit

    def paced_merge(main, side, min_lead=0):
        """Interleave side units evenly into main units; no side unit
        before main index min_lead (so deferred writes they depend on have
        been emitted)."""
        if not side:
            for u in main:
                u()
            return
        n, m = len(main), len(side)
        k = 0
        for i, u in enumerate(main):
            u()
            if i + 1 < max(min_lead, 1):
                continue
            want = (i + 1) * m // n
            while k < want:
                side[k]()
                k += 1
        while k < m:
            side[k]()
            k += 1

    # ---- schedule ----
    st0 = prep_qkv(0)
    load_xt_quarter(0, nc.sync)
    load_wv(nc.scalar)
    load_xt_quarter(1, nc.sync)
    load_xt_quarter(2, nc.scalar)
    load_xt_quarter(3, nc.scalar)
    # quarter-paced qkv(pair0) + v production covering the input DMA ramp;
    # pair-0 attention q-blocks run one quarter behind their inputs
    yts = []
    pair_state = [st0]
    yt0 = po_yt.tile([128, T], F16, tag="yT", name="yt")
    yts.append(yt0)
    attn0 = attn_units(st0, yt0)   # ascending: qb0(4), qb1(8), qb2(12), qb3(16)
    qb_at = [0, 0, 4, 12, 24]
    for q in range(4):
        qkv_unit(st0, q)
        for tt in range(4 * q, 4 * q + 4):
            v_unit(tt)
        for u in attn0[qb_at[q]:qb_at[q + 1]]:
            u()
    for p in range(NPAIR):
        yt = yts[0] if p == 0 else po_yt.tile([128, T], F16, tag="yT",
                                              name="yt")
        if p > 0:
            yts.append(yt)
        side = []
        if p + 1 < NPAIR:
            stn = prep_qkv(p + 1)
            pair_state.append(stn)
            side = [(lambda s=stn, tb=tb: qkv_unit(s, tb)) for tb in range(NTB)]
        if p == 2:
            side.append(load_wp)
        attn = attn0[24:] if p == 0 else attn_units(pair_state[p], yt)
        if p < NPAIR - 1:
            paced_merge(attn, side)
        else:
            # pair 3 runs q-blocks descending; projection for a q-block's
            # four t-tiles unlocks as soon as that q-block finishes.
            qb_sizes = [16, 12, 8, 4]
            qb_tts = [(12, 16), (8, 12), (4, 8), (0, 4)]
            pos = 0
            proj_ready = []
            for i, sz in enumerate(qb_sizes):
                chunk = attn[pos:pos + sz]
                pos += sz
                paced_merge(chunk, proj_ready, min_lead=4)
                proj_ready = [proj_unit(tt, yts)
                              for tt in range(*qb_tts[i])]
            for u in attn[pos:]:
                u()          # trailing flush: finishes qb0
            for u in proj_ready:
                u()

    ctx.close()


_CACHE = {}


def _build():
    if "nc" in _CACHE:
        return _CACHE["nc"]
    nc = bacc.Bacc("TRN2", target_bir_lowering=False, debug=False,
                   enable_asserts=True, num_devices=N_CORES)
    aps = {
        "xt": nc.dram_tensor("xt", [512, 4096], F16,
                             kind="ExternalInput").ap(),
        "wq": nc.dram_tensor("wq", [512, CCH * 256], F16,
                             kind="ExternalInput").ap(),
        "wva": nc.dram_tensor("wva", [128, CCH * VW], F16,
                              kind="ExternalInput").ap(),
        "bq": nc.dram_tensor("bq", [F, 1], F32, kind="ExternalInput").ap(),
        "bk": nc.dram_tensor("bk", [F, 1], F32, kind="ExternalInput").ap(),
        "bva2": nc.dram_tensor("bva2", [1, VW], F32, kind="ExternalInput").ap(),
        "wp": nc.dram_tensor("wp", [128, 8 * 512], F16,
                             kind="ExternalInput").ap(),
        "mask01": nc.dram_tensor("mask01", [128, 512], F16,
                                 kind="ExternalInput").ap(),
        "out": nc.dram_tensor("out", [T, C], F16,
                              kind="ExternalOutput").ap(),
    }
    with tile.TileContext(nc) as tc:
        _emit(tc, aps)
    nc.compile()
    _CACHE["nc"] = nc
    return nc


def _make_in_maps(x, Wqkv, bqkv, Wproj):
    x = np.asarray(x, dtype=np.float32)
    Wqkv = np.asarray(Wqkv, dtype=np.float32)
    bqkv = np.asarray(bqkv, dtype=np.float32)
    Wproj = np.asarray(Wproj, dtype=np.float32)

    # 0/1 causal mask: visible (1) when tq-within-block >= tk-partition
    p_idx = np.arange(128)[:, None]
    u_idx = np.arange(512)[None, :]
    mask01 = (u_idx >= p_idx).astype(np.float16)

    in_maps = []
    for core in range(N_CORES):
        b, g = divmod(core, 2)
        q0, k0, v0 = 512 * g, C + 512 * g, 2 * C + 512 * g
        wva = np.zeros((C, VW), dtype=np.float32)
        bva = np.zeros((1, VW), dtype=np.float32)
        for h in range(NH):
            src = v0 + D * h
            dst = 65 * h
            # per-head layout [v(64), one]
            wva[:, dst:dst + 64] = Wqkv[:, src:src + 64]
            bva[0, dst:dst + 64] = bqkv[src:src + 64]
            bva[0, dst + 64] = 1.0
        # xq[q*128+p, c*512+j] = x[b][q*512+j, c*128+p]
        xq = (x[b].astype(np.float16).T          # [C, T]
              .reshape(CCH, 128, 4, 512)         # c, p, q, j
              .transpose(2, 1, 0, 3)             # q, p, c, j
              .reshape(512, 4096))
        # wqk[pair*128+p, c*256+j] = (wq|wk)[c*128+p, pair*128+j']
        wq_ = Wqkv[:, q0:q0 + F].astype(np.float16).reshape(CCH, 128, 4, 128)
        wk_ = Wqkv[:, k0:k0 + F].astype(np.float16).reshape(CCH, 128, 4, 128)
        wqk = np.concatenate([wq_, wk_], axis=3)  # c, p, pair, 256
        wqk = wqk.transpose(2, 1, 0, 3).reshape(512, CCH * 256)
        # wvp[p, c*520+j] = wva[c*128+p, j]
        wvp = (wva.astype(np.float16).reshape(CCH, 128, VW)
               .transpose(1, 0, 2).reshape(128, CCH * VW))
        # wpp[p, (pr*2+cb)*512+j] = Wproj[512g + pr*128+p, cb*512+j]
        wpp = (Wproj[512 * g:512 * g + F, :].astype(np.float16)
               .reshape(4, 128, 2, 512).transpose(1, 0, 2, 3)
               .reshape(128, 8 * 512))
        in_maps.append({
            "xt": np.ascontiguousarray(xq),
            "wq": np.ascontiguousarray(wqk),
            "wva": np.ascontiguousarray(wvp),
            "bq": np.ascontiguousarray(bqkv[q0:q0 + F].reshape(F, 1) * 0.125),
            "bk": np.ascontiguousarray(bqkv[k0:k0 + F].reshape(F, 1)),
            "bva2": bva,
            "wp": np.ascontiguousarray(wpp),
            "mask01": mask01,
        })
    return in_maps


def run_sharded(x, Wqkv, bqkv, Wproj, bproj, trace=False):
    nc = _build()
    in_maps = _make_in_maps(x, Wqkv, bqkv, Wproj)
    res = run_bass_kernel_spmd(nc, in_maps, core_ids=list(range(N_CORES)),
                               trace=trace)
    bproj = np.asarray(bproj, dtype=np.float32)
    out = np.empty((B, T, C), dtype=np.float32)
    for b in range(B):
        out[b] = (bproj[None, :]
                  + res.results[2 * b]["out"].astype(np.float32)
                  + res.results[2 * b + 1]["out"].astype(np.float32))
    return out, res


def kernel(x, Wqkv, bqkv, Wproj, bproj):
    out, _ = run_sharded(x, Wqkv, bqkv, Wproj, bproj, trace=False)
    return out
